# revision 1
# baseline (speedup 1.0000x reference)
"""MoE layer (16 experts, top-2, capacity 1280) on 8 Trainium2 cores.

Strategy: expert-parallel. Host does the gate + routing (sigmoid top-2,
capacity ranking) and builds the padded per-expert token buffers — this is
the "dispatch" part of the sharding. Each core runs the dense two-matmul
FFN for 2 experts in fp32r (full-rate fp32 with 11-bit mantissa) with
transposed activations so no on-device transposes are needed:

  H^T[f,c]  = sum_d W1[d,f]·X^T[d,c]   (lhsT = natural-layout W1 tile)
  Y^T[d,c]  = sum_f W2[f,d]·gelu(H^T)  (lhsT = natural-layout W2 tile)

Host then gathers Y^T, un-pads, applies the top-2 combine weights.
"""

import numpy as np

# Problem constants (hardcoded per contract; kernel.py must be self-contained).
D_MODEL = 1024
D_FF = 2048
NUM_EXPERTS = 16
TOP_K = 2
CAPACITY_FACTOR = 1.25
BATCH, SEQ = 4, 2048
T = BATCH * SEQ
CAPACITY = int(CAPACITY_FACTOR * T * TOP_K / NUM_EXPERTS)  # 1280

N_CORES = 8
EPC = NUM_EXPERTS // N_CORES  # experts per core = 2
P = 128
CN = 320  # token-chunk (PSUM free dim); 1280 = 4 * 320
ND = D_MODEL // P  # 8
NF = D_FF // P  # 16
NCH = CAPACITY // CN  # 4

_CACHE: dict = {}
LAST_RESULTS = None  # BassKernelResults of the most recent run (for test.py)


def _round_fp32r(a: np.ndarray) -> np.ndarray:
    """Round fp32 to fp32r: 11-bit mantissa (low 12 bits zero), RNE."""
    u = np.ascontiguousarray(a, dtype=np.float32).view(np.uint32)
    lsb = (u >> 12) & 1
    r = (u + 0x7FF + lsb) & np.uint32(0xFFFFF000)
    return r.view(np.float32)


def _build_nc():
    import concourse.bacc as bacc
    import concourse.mybir as mybir
    import concourse.tile as tile

    F32 = mybir.dt.float32
    F32R = mybir.dt.float32r
    GELU = mybir.ActivationFunctionType.Gelu

    nc = bacc.Bacc()
    xt_d = nc.declare_dram_parameter("xt", [EPC, D_MODEL, CAPACITY], F32R, isOutput=False)
    w1_d = nc.declare_dram_parameter("w1", [EPC, D_MODEL, D_FF], F32R, isOutput=False)
    w2_d = nc.declare_dram_parameter("w2", [EPC, D_FF, D_MODEL], F32R, isOutput=False)
    yt_d = nc.declare_dram_parameter("yt", [EPC, D_MODEL, CAPACITY], F32, isOutput=True)

    with tile.TileContext(nc) as tc:
        with (
            tc.tile_pool(name="sbuf", bufs=1) as pool,
            tc.tile_pool(name="psum", bufs=1, space="PSUM") as psum,
        ):
            for e in range(EPC):
                w1_t = [
                    pool.tile([P, D_FF], F32R, tag=f"w1_{i}", name=f"w1_{e}_{i}")
                    for i in range(ND)
                ]
                w2_t = [
                    pool.tile([P, D_MODEL], F32R, tag=f"w2_{i}", name=f"w2_{e}_{i}")
                    for i in range(NF)
                ]
                for i in range(ND):
                    nc.sync.dma_start(out=w1_t[i][:], in_=w1_d[e, i * P : (i + 1) * P, :])
                for i in range(NF):
                    nc.sync.dma_start(out=w2_t[i][:], in_=w2_d[e, i * P : (i + 1) * P, :])

                for ci in range(NCH):
                    c0 = ci * CN
                    xt_t = [
                        pool.tile([P, CN], F32R, tag=f"xt_{i}", name=f"xt_{e}_{ci}_{i}", bufs=2)
                        for i in range(ND)
                    ]
                    for i in range(ND):
                        nc.sync.dma_start(
                            out=xt_t[i][:], in_=xt_d[e, i * P : (i + 1) * P, c0 : c0 + CN]
                        )

                    h_t = [
                        pool.tile([P, CN], F32R, tag=f"h_{i}", name=f"h_{e}_{ci}_{i}")
                        for i in range(NF)
                    ]
                    for fj in range(NF):
                        acc = psum.tile(
                            [P, CN], F32, tag="mm1", name=f"acc1_{e}_{ci}_{fj}", bufs=3
                        )
                        for kj in range(ND):
                            nc.tensor.matmul(
                                acc[:],
                                w1_t[kj][:, fj * P : (fj + 1) * P],
                                xt_t[kj][:],
                                start=(kj == 0),
                                stop=(kj == ND - 1),
                            )
                        nc.scalar.activation(h_t[fj][:], acc[:], GELU)

                    for mj in range(ND):
                        acc2 = psum.tile(
                            [P, CN], F32, tag="mm2", name=f"acc2_{e}_{ci}_{mj}", bufs=3
                        )
                        for fj in range(NF):
                            nc.tensor.matmul(
                                acc2[:],
                                w2_t[fj][:, mj * P : (mj + 1) * P],
                                h_t[fj][:],
                                start=(fj == 0),
                                stop=(fj == NF - 1),
                            )
                        out_t = pool.tile(
                            [P, CN], F32, tag="out", name=f"out_{e}_{ci}_{mj}", bufs=4
                        )
                        nc.vector.tensor_copy(out_t[:], acc2[:])
                        nc.sync.dma_start(
                            out=yt_d[e, mj * P : (mj + 1) * P, c0 : c0 + CN], in_=out_t[:]
                        )

    nc.finalize()
    return nc


def _route(x, gate_w):
    """Replicate the reference routing exactly (numpy fp32)."""
    xf = x.reshape(T, D_MODEL).astype(np.float32, copy=False)
    logits = xf @ gate_w.T.astype(np.float32, copy=False)  # [T, E]
    probs = 1.0 / (1.0 + np.exp(-logits, dtype=np.float32))
    # top-2, ties -> lower index first (matches jax.lax.top_k)
    sel = np.argsort(-probs, axis=1, kind="stable")[:, :TOP_K].astype(np.int32)
    top_vals = np.take_along_axis(probs, sel, axis=1)
    weights = top_vals / (top_vals.sum(-1, keepdims=True) + np.float32(1e-6))

    flat_sel = sel.reshape(-1)  # [T*K]
    onehot = np.zeros((T * TOP_K, NUM_EXPERTS), dtype=np.int32)
    onehot[np.arange(T * TOP_K), flat_sel] = 1
    ranks = np.take_along_axis(
        np.cumsum(onehot, axis=0), flat_sel[:, None], axis=1
    )[:, 0] - 1
    valid = ranks < CAPACITY
    sentinel = NUM_EXPERTS * CAPACITY
    slot = np.where(valid, flat_sel * CAPACITY + ranks, sentinel).astype(np.int64)
    return xf, weights.reshape(-1), slot, valid


def kernel(x, gate_w, w1, w2):
    global LAST_RESULTS
    import os

    from concourse.bass_utils import run_bass_kernel_spmd

    xf, wflat, slot, valid = _route(np.asarray(x), np.asarray(gate_w))

    sentinel = NUM_EXPERTS * CAPACITY
    padded = np.zeros((sentinel, D_MODEL), dtype=np.float32)
    tok = np.nonzero(valid)[0]
    padded[slot[tok]] = xf[tok // TOP_K]

    # [E, d, C] transposed per-expert buffers, rounded to fp32r
    xt_all = _round_fp32r(
        np.ascontiguousarray(
            padded.reshape(NUM_EXPERTS, CAPACITY, D_MODEL).transpose(0, 2, 1)
        )
    )
    w1r = _round_fp32r(np.asarray(w1))
    w2r = _round_fp32r(np.asarray(w2))

    if "nc" not in _CACHE:
        _CACHE["nc"] = _build_nc()
    nc = _CACHE["nc"]

    in_maps = [
        {
            "xt": xt_all[c * EPC : (c + 1) * EPC],
            "w1": w1r[c * EPC : (c + 1) * EPC],
            "w2": w2r[c * EPC : (c + 1) * EPC],
        }
        for c in range(N_CORES)
    ]
    trace = bool(int(os.environ.get("MOE_TRACE", "0")))
    res = run_bass_kernel_spmd(nc, in_maps, list(range(N_CORES)), trace=trace)
    LAST_RESULTS = res

    yt = np.concatenate([res.results[c]["yt"] for c in range(N_CORES)], axis=0)
    out_padded = np.ascontiguousarray(yt.transpose(0, 2, 1)).reshape(sentinel, D_MODEL)

    gathered = out_padded[np.minimum(slot, sentinel - 1)]
    gathered[~valid] = 0.0
    out = (gathered * wflat[:, None]).reshape(T, TOP_K, D_MODEL).sum(axis=1)
    return out.reshape(BATCH, SEQ, D_MODEL).astype(np.float32, copy=False)
